# revision 1
# baseline (speedup 1.0000x reference)
"""TRN2 Bass kernel for nn_DS_Block (sparse attention block).

Pipeline per sample (b=32 sharded 4-per-core over 8 cores):
  x [128, 4096] --1x1conv+3tap-dwconv (folded: 3 shifted fp32r matmuls)-->
  v [128(o), 4096]           (natural layout, for a@v)
  qkT chunks [128(n), 256(o)] (transposed layout, for attention)
  G = qkT^T qkT gram accumulation -> attn blocks + q/k norms (diagonals)
  rank-based 4-way top-k masked softmax on 32x32 blocks (DVE/ACT)
  out = (proj_w @ A_blkdiag) @ v + proj_b   (P^T fused, fp32r matmuls)
"""
import os
import sys
import json
from contextlib import ExitStack

sys.path.insert(0, "/opt/trn_rl_repo")

import numpy as np
import concourse.bass as bass
import concourse.mybir as mybir
import concourse.tile as tile
from concourse.bass_utils import run_bass_kernel_spmd

F32 = mybir.dt.float32
F32R = mybir.dt.float32r
AF = mybir.ActivationFunctionType
OP = mybir.AluOpType

B, CDIM, N = 32, 128, 4096
HEADS, C = 4, 32
NCORES = 8
NSAMP = B // NCORES          # 4 samples per core
NT = N // 512                # 8 n-tiles of 512
NCH = N // 128               # 32 chunks of 128
TOPKS = [C // 2, (2 * C) // 3, (3 * C) // 4, (4 * C) // 5]  # 16,21,24,25


def _fix_sync_waits(bir: dict, max_waits: int = 1) -> dict:
    """This walrus build rejects >1 sem-wait per instruction; split the
    excess into standalone EventSemaphore waits on the same engine queue."""
    for f in bir.get("functions", []):
        for blk in f.get("blocks", []):
            out = []
            for inst in blk.get("instructions", []):
                si = inst.get("sync_info") or {}
                waits = si.get("on_wait") or []
                if len(waits) > max_waits:
                    extra, keep = waits[:-max_waits], waits[-max_waits:]
                    for j, w in enumerate(extra):
                        out.append({
                            "debug": inst.get("debug", 0),
                            "engine": inst["engine"],
                            "ins": [], "outs": [],
                            "name": f"{inst['name']}-xw{j}",
                            "opcode": "EventSemaphore",
                            "sync_info": {"on_update": [], "on_wait": [w]},
                        })
                    si["on_wait"] = keep
                    inst["sync_info"] = si
                out.append(inst)
            blk["instructions"] = out
    return bir


def _patch_nc(nc):
    orig = nc.to_json_bytes

    def fixed():
        return json.dumps(_fix_sync_waits(json.loads(orig()))).encode()

    nc.to_json_bytes = fixed
    return nc


def build_program(debug: bool = False):
    nc = bass.Bass("TRN2", target_bir_lowering=False, num_devices=NCORES)

    x_d = nc.dram_tensor("x", [NSAMP, CDIM, N], F32R, kind="ExternalInput")
    out_d = nc.dram_tensor("out", [NSAMP, CDIM, N], F32, kind="ExternalOutput")
    w3qk_d = nc.dram_tensor("w3qk", [3, CDIM, 256], F32R, kind="ExternalInput")
    w3v_d = nc.dram_tensor("w3v", [3, CDIM, CDIM], F32R, kind="ExternalInput")
    bqk2_d = nc.dram_tensor("bqk2", [CDIM, 512], F32, kind="ExternalInput")
    eqkL_d = nc.dram_tensor("eqkL", [128, 256], F32, kind="ExternalInput")
    eqkR_d = nc.dram_tensor("eqkR", [128, 256], F32, kind="ExternalInput")
    bv_d = nc.dram_tensor("bv", [CDIM, 1], F32, kind="ExternalInput")
    evL_d = nc.dram_tensor("evL", [CDIM, 1], F32, kind="ExternalInput")
    evR_d = nc.dram_tensor("evR", [CDIM, 1], F32, kind="ExternalInput")
    pwT_d = nc.dram_tensor("pwT", [CDIM, CDIM], F32, kind="ExternalInput")
    pb_d = nc.dram_tensor("pb", [CDIM, 1], F32, kind="ExternalInput")
    id128_d = nc.dram_tensor("id128", [CDIM, CDIM], F32, kind="ExternalInput")
    ones32_d = nc.dram_tensor("ones32", [1, 32], F32, kind="ExternalInput")
    lntemp_d = nc.dram_tensor("lntemp", [CDIM, 1], F32, kind="ExternalInput")
    wS_d = nc.dram_tensor("wS", [CDIM, 4], F32, kind="ExternalInput")
    dbg = {}
    if debug:
        for nm, shp in [("d_v", [CDIM, N]), ("d_qkt", [CDIM, 512]),
                        ("d_attn", [CDIM, 32]), ("d_rank", [CDIM, 32]),
                        ("d_A", [CDIM, 32]), ("d_e", [CDIM, 32]),
                        ("d_rq", [CDIM, 1]), ("d_rk", [CDIM, 1]),
                        ("d_RK", [CDIM, 32]), ("d_S", [CDIM, 4])]:
            dbg[nm] = nc.dram_tensor(nm, shp, F32, kind="ExternalOutput")

    with tile.TileContext(nc) as tc, ExitStack() as es:
        wp = es.enter_context(tc.tile_pool(name="wp", bufs=1))
        xp = es.enter_context(tc.tile_pool(name="xp", bufs=3))
        vp = es.enter_context(tc.tile_pool(name="vp", bufs=2))
        qkp = es.enter_context(tc.tile_pool(name="qkp", bufs=6))
        ap = es.enter_context(tc.tile_pool(name="ap", bufs=2))
        op_ = es.enter_context(tc.tile_pool(name="op", bufs=2))
        ps_v = es.enter_context(tc.tile_pool(name="ps_v", bufs=2, space="PSUM"))
        ps_qk = es.enter_context(tc.tile_pool(name="ps_qk", bufs=3, space="PSUM"))
        ps_g = es.enter_context(tc.tile_pool(name="ps_g", bufs=2, space="PSUM"))
        ps_s = es.enter_context(tc.tile_pool(name="ps_s", bufs=1, space="PSUM"))

        # ---- constants ----
        w3qk = wp.tile([CDIM, 3, 256], F32R)
        w3v = wp.tile([CDIM, 3, CDIM], F32R)
        bqk2 = wp.tile([CDIM, 512], F32)
        eqkL = wp.tile([128, 256], F32)
        eqkR = wp.tile([128, 256], F32)
        bv = wp.tile([CDIM, 1], F32)
        evL = wp.tile([CDIM, 1], F32)
        evR = wp.tile([CDIM, 1], F32)
        pwT = wp.tile([CDIM, CDIM], F32)
        pb = wp.tile([CDIM, 1], F32)
        id128 = wp.tile([CDIM, CDIM], F32)
        ones32 = wp.tile([1, 32], F32)
        lntemp = wp.tile([CDIM, 1], F32)
        wS = wp.tile([CDIM, 4], F32)
        for t in range(3):
            nc.sync.dma_start(w3qk[:, t, :], w3qk_d[t])
            nc.sync.dma_start(w3v[:, t, :], w3v_d[t])
        for t_, d_ in [(bqk2, bqk2_d),
                       (eqkL, eqkL_d), (eqkR, eqkR_d), (bv, bv_d),
                       (evL, evL_d), (evR, evR_d), (pwT, pwT_d), (pb, pb_d),
                       (id128, id128_d), (ones32, ones32_d),
                       (lntemp, lntemp_d), (wS, wS_d)]:
            nc.sync.dma_start(t_[:], d_[:])

        for s in range(NSAMP):
            # ---- load x with 1-col zero halo on each side ----
            xt = xp.tile([CDIM, N + 2], F32R, tag="x")
            if s < 3:  # halo cols persist across slot reuse (bufs=3)
                xf = xt[:].bitcast(F32)
                nc.vector.memset(xf[:, 0:1], 0.0)
                nc.vector.memset(xf[:, N + 1:N + 2], 0.0)
            for q in range(8):
                nc.sync.dma_start(xt[:, 1 + q * 512:1 + (q + 1) * 512],
                                  x_d[s, :, q * 512:(q + 1) * 512])

            # ---- qkT-pass + gram accumulation ----
            G01 = ps_g.tile([CDIM, 512], F32, tag="g")  # [QtQ|QtK] 0:256, [KtQ|KtK] 256:512
            for cp in range(0 if os.environ.get("ABL_NOQKT") else NCH // 2):
                qkps = ps_qk.tile([CDIM, 512], F32, tag="qk")
                for half in range(2):
                    ch = cp * 2 + half
                    for t in range(3):
                        nc.tensor.matmul(
                            qkps[:, half * 256:(half + 1) * 256],
                            xt[:, ch * 128 + t: ch * 128 + t + 128],
                            w3qk[:, t, :], start=(t == 0), stop=(t == 2))
                qkt = qkp.tile([CDIM, 512], F32R, tag="qkt")
                nc.vector.scalar_tensor_tensor(qkt[:], qkps[:], 1.0, bqk2[:],
                                               OP.mult, OP.add)
                if cp == 0:
                    nc.vector.tensor_tensor(qkt[:, 0:256], qkt[:, 0:256],
                                            eqkL[:], OP.add)
                if cp == NCH // 2 - 1:
                    nc.vector.tensor_tensor(qkt[:, 256:512],
                                            qkt[:, 256:512], eqkR[:],
                                            OP.add)
                if debug and s == 0 and cp == 0:
                    nc.sync.dma_start(dbg["d_qkt"][:], qkt[:].bitcast(F32))
                for half in range(2):
                    ch = cp * 2 + half
                    off = half * 256
                    nc.tensor.matmul(G01[:, 0:256], qkt[:, off:off + 128],
                                     qkt[:, off:off + 256],
                                     start=(ch == 0), stop=(ch == NCH - 1))
                    nc.tensor.matmul(G01[:, 256:512],
                                     qkt[:, off + 128:off + 256],
                                     qkt[:, off:off + 256],
                                     start=(ch == 0), stop=(ch == NCH - 1))

            # ---- v-pass: v[o, n] = sum_t W3v_t^T.T @ x_shift ----
            vt = vp.tile([CDIM, N], F32R, tag="v")
            for nt in range(0 if os.environ.get("ABL_NOV") else NT):
                vps = ps_v.tile([CDIM, 512], F32, tag="vv")
                for t in range(3):
                    nc.tensor.matmul(vps[:], w3v[:, t, :],
                                     xt[:, nt * 512 + t: nt * 512 + t + 512],
                                     start=(t == 0), stop=(t == 2))
                nc.scalar.activation(vt[:, nt * 512:(nt + 1) * 512], vps[:],
                                     AF.Identity, bias=bv[:])
            nc.vector.tensor_tensor(vt[:, 0:1], vt[:, 0:1], evL[:], OP.add)
            nc.vector.tensor_tensor(vt[:, N - 1:N], vt[:, N - 1:N], evR[:],
                                    OP.add)
            if debug and s == 0:
                nc.sync.dma_start(dbg["d_v"][:], vt[:].bitcast(F32))

            # ---- attention phase (per sample, [128, 32] tiles) ----
            ABL_NOPHASE = bool(os.environ.get("ABL_NOPHASE"))
            sm = ps_s.tile([CDIM, 512], F32, tag="sm")  # rkT 0:128, RK 128:160, PT 256:384
            attn_raw = ap.tile([CDIM, 32], F32, tag="attn")
            for h in range(0 if ABL_NOPHASE else HEADS):
                nc.scalar.copy(attn_raw[32 * h:32 * (h + 1), :],
                               G01[32 * h:32 * (h + 1),
                                   128 + 32 * h:128 + 32 * (h + 1)])
            A = ap.tile([CDIM, 32], F32, tag="A")
            if ABL_NOPHASE:
                nc.vector.memset(A[:], 0.03)
            else:
                qsq = ap.tile([CDIM, 1], F32, tag="qsq")
                ksq = ap.tile([CDIM, 1], F32, tag="ksq")
                scr = ap.tile([CDIM, 128], F32, tag="scr")
                nc.vector.tensor_tensor(scr[:], G01[:, 384:512], id128[:], OP.mult)
                nc.vector.tensor_reduce(ksq[:], scr[:], mybir.AxisListType.X,
                                        OP.add)
                nc.vector.tensor_tensor(scr[:], G01[:, 0:128], id128[:], OP.mult)
                nc.vector.tensor_reduce(qsq[:], scr[:], mybir.AxisListType.X,
                                        OP.add)
                rq = ap.tile([CDIM, 1], F32, tag="rq")
                rk = ap.tile([CDIM, 1], F32, tag="rk")
                nc.scalar.activation(rk[:], ksq[:], AF.Ln)
                nc.scalar.activation(rk[:], rk[:], AF.Exp, scale=-0.5)
                nc.scalar.activation(rq[:], qsq[:], AF.Ln)
                nc.scalar.activation(rq[:], rq[:], AF.Exp, bias=lntemp[:],
                                     scale=-0.5)
                # broadcast rk over its head-block columns: transpose + 4 K=1 mms
                nc.tensor.transpose(sm[0:1, 0:128], rk[:], id128[:])
                rkrow = ap.tile([1, 128], F32, tag="rkrow")
                nc.scalar.copy(rkrow[:], sm[0:1, 0:128])
                for h in range(HEADS):
                    nc.tensor.matmul(sm[32 * h:32 * (h + 1), 128:160], ones32[:],
                                     rkrow[0:1, 32 * h:32 * (h + 1)],
                                     tile_position=(0, 32 * h))
                attn_s = ap.tile([CDIM, 32], F32, tag="attn_s")
                nc.vector.scalar_tensor_tensor(attn_s[:], attn_raw[:], rq[:],
                                               sm[:, 128:160], OP.mult, OP.mult)
                if debug and s == 0:
                    nc.sync.dma_start(dbg["d_attn"][:], attn_s[:])
                    nc.sync.dma_start(dbg["d_rq"][:], rq[:])
                    nc.sync.dma_start(dbg["d_rk"][:], rk[:])
                    rkb = ap.tile([CDIM, 32], F32, tag="rkb")
                    nc.vector.tensor_copy(rkb[:], sm[:, 128:160])
                    nc.sync.dma_start(dbg["d_RK"][:], rkb[:])
                # e = exp(attn_s - rowmax)
                nmx = ap.tile([CDIM, 1], F32, tag="nmx")
                nc.vector.tensor_reduce(nmx[:], attn_s[:], mybir.AxisListType.X,
                                        OP.max, negate=True)
                e = ap.tile([CDIM, 32], F32, tag="e")
                nc.scalar.activation(e[:], attn_s[:], AF.Exp, bias=nmx[:])
                # ranks: rank[c,d] = #{d' : attn[c,d'] > attn[c,d]}
                rank = ap.tile([CDIM, 32], F32, tag="rank")
                cmp = ap.tile([CDIM, C, C], F32, tag="cmp")
                nc.vector.tensor_tensor(
                    cmp[:], attn_s[:, :, None].to_broadcast((CDIM, C, C)),
                    attn_s[:, None, :].to_broadcast((CDIM, C, C)), OP.is_lt)
                nc.vector.tensor_reduce(rank[:], cmp[:],
                                        mybir.AxisListType.X, OP.add)
                # masked sums S_i and masked-e tiles
                S = ap.tile([CDIM, 4], F32, tag="S")
                me = [ap.tile([CDIM, 32], F32, tag=f"me{i}", name=f"me{i}") for i in range(4)]
                for i, kk in enumerate(TOPKS):
                    nc.vector.scalar_tensor_tensor(me[i][:], rank[:], float(kk),
                                                   e[:], OP.is_lt, OP.mult,
                                                   accum_out=S[:, i:i + 1])
                R = ap.tile([CDIM, 4], F32, tag="R")
                nc.vector.reciprocal(R[:], S[:])
                nc.vector.tensor_tensor(R[:], R[:], wS[:], OP.mult)
                A = ap.tile([CDIM, 32], F32, tag="A")
                nc.vector.tensor_scalar_mul(A[:], me[0][:], R[:, 0:1])
                for i in range(1, 4):
                    nc.vector.scalar_tensor_tensor(A[:], me[i][:], R[:, i:i + 1],
                                                   A[:], OP.mult, OP.add)
                if debug and s == 0:
                    nc.sync.dma_start(dbg["d_rank"][:], rank[:])
                    nc.sync.dma_start(dbg["d_A"][:], A[:])
                    nc.sync.dma_start(dbg["d_e"][:], e[:])
                    nc.sync.dma_start(dbg["d_S"][:], S[:])
            # P^T = A_blkdiag^T @ proj_w^T  (lhsT = block-diag of A)
            BD = ap.tile([CDIM, CDIM], F32, tag="BD")
            if s < 2:  # off-diag zeros persist across slot reuse (bufs=2)
                nc.vector.memset(BD[:], 0.0)
            for h in range(HEADS):
                nc.scalar.copy(BD[32 * h:32 * (h + 1),
                                  32 * h:32 * (h + 1)],
                               A[32 * h:32 * (h + 1), :])
            nc.tensor.matmul(sm[:, 256:384], BD[:], pwT[:])
            PT = ap.tile([CDIM, CDIM], F32R, tag="PT")
            nc.scalar.copy(PT[:], sm[:, 256:384])
            # av + out
            ot = op_.tile([CDIM, N], F32, tag="out")
            for nt in range(0 if os.environ.get("ABL_NOAV") else NT):
                avps = ps_v.tile([CDIM, 512], F32, tag="vv")
                nc.tensor.matmul(avps[:], PT[:],
                                 vt[:, nt * 512:(nt + 1) * 512])
                nc.scalar.activation(ot[:, nt * 512:(nt + 1) * 512], avps[:],
                                     AF.Identity, bias=pb[:])
            for q in range(8):
                nc.sync.dma_start(out_d[s, :, q * 512:(q + 1) * 512],
                                  ot[:, q * 512:(q + 1) * 512])

    _patch_nc(nc)
    return nc


_NC_CACHE = {}


def _get_nc(debug=False):
    key = bool(debug)
    if key not in _NC_CACHE:
        _NC_CACHE[key] = build_program(debug=key)
    return _NC_CACHE[key]


def make_inputs(x, qkv_w, qkv_b, dw_w, dw_b, proj_w, proj_b, temperature,
                attn_w):
    """Host-side weight prep -> per-core input maps."""
    x = np.ascontiguousarray(np.asarray(x, np.float32)[:, :, :, 0])
    qkv_w = np.asarray(qkv_w, np.float32)
    qkv_b = np.asarray(qkv_b, np.float32)
    dw_w = np.asarray(dw_w, np.float32)
    dw_b = np.asarray(dw_b, np.float32)
    proj_w = np.asarray(proj_w, np.float32)
    proj_b = np.asarray(proj_b, np.float32)
    temperature = np.asarray(temperature, np.float32).reshape(HEADS)
    attn_w = np.asarray(attn_w, np.float32)

    dwk = dw_w[:, 0, :, 1]                       # [384, 3]
    w3qk = np.stack([(qkv_w[:256] * dwk[:256, t:t + 1]).T.copy()
                     for t in range(3)])          # [3, 128, 256]
    w3v = np.stack([(qkv_w[256:] * dwk[256:, t:t + 1]).T.copy()
                    for t in range(3)])           # [3, 128, 128]
    bqk = qkv_b[:256] * dwk[:256].sum(1) + dw_b[:256]          # [256]
    bqk2 = np.tile(np.concatenate([bqk, bqk])[None, :], (CDIM, 1))  # [128,512]
    eqkL = np.zeros((128, 256), np.float32)
    eqkL[0] = -qkv_b[:256] * dwk[:256, 0]
    eqkR = np.zeros((128, 256), np.float32)
    eqkR[127] = -qkv_b[:256] * dwk[:256, 2]
    bv = (qkv_b[256:] * dwk[256:].sum(1) + dw_b[256:])[:, None]  # [128,1]
    evL = (-qkv_b[256:] * dwk[256:, 0])[:, None]
    evR = (-qkv_b[256:] * dwk[256:, 2])[:, None]
    pwT = proj_w.T.copy()                        # [c, o]
    pb = proj_b[:, None].copy()
    id128 = np.eye(CDIM, dtype=np.float32)
    ones32 = np.ones((1, 32), np.float32)
    lntemp = np.repeat(np.log(np.maximum(temperature, 1e-30)), C)[:, None]
    lntemp = np.ascontiguousarray(lntemp, np.float32)
    wS = np.tile(attn_w[None, :], (CDIM, 1))

    const = dict(w3qk=w3qk, w3v=w3v, bqk2=bqk2.astype(np.float32),
                 eqkL=eqkL.astype(np.float32), eqkR=eqkR.astype(np.float32),
                 bv=bv.astype(np.float32), evL=evL.astype(np.float32),
                 evR=evR.astype(np.float32), pwT=pwT, pb=pb, id128=id128,
                 ones32=ones32, lntemp=lntemp, wS=wS.astype(np.float32))
    maps = []
    for i in range(NCORES):
        m = dict(const)
        m["x"] = np.ascontiguousarray(x[i * NSAMP:(i + 1) * NSAMP])
        maps.append(m)
    return maps


def kernel(**inputs):
    nc = _get_nc(debug=False)
    maps = make_inputs(**inputs)
    res = run_bass_kernel_spmd(nc, maps, list(range(NCORES)))
    outs = [res.results[i]["out"] for i in range(NCORES)]
    full = np.concatenate(outs, axis=0)          # [32, 128, 4096]
    return full[:, :, :, None].astype(np.float32)

